# revision 23
# baseline (speedup 1.0000x reference)
"""Multi-head self-attention (B=2, S=2048, E=1024, H=16) on 8 Trainium2 cores.

Sharding: 2D (batch x head-group). Core c handles batch b = c // 4 and head
group g = c % 4 (4 heads, 256 embed columns). Each core computes its QKV
projection slices, fused attention for its 4 heads, and a partial output
projection (attn_g @ Wo[g_slice]); the host sums the 4 partials per batch
(the head-concat contraction) and stacks the 2 batches.

Device structure (v2 — ScalarE-exp / TensorE co-bound pipeline):
  - all input x spans are loaded ONCE (each [128,KC,span] slice is shared by
    the c=0 and c=1 column groups of its projection) through the single
    hardware DMA FIFO in consumption-priority order, so the first logits
    fire ~3MB into the stream instead of after the whole input set
  - attention runs in 2-jc batches: [4 row-packed K=64 logits MMs]
    [2 exps] [4 PV MMs lagging 2 jc] [fillers]; the PV lag keeps the PE
    FIFO from ever waiting on ScalarE, and batching halves the PE
    64-row/128-row tiling-mode switches
  - projections (K/Q column groups, V chunks) and the partial output
    projection ride as fillers in the batch slots, scheduled so every
    dependency (KT span by jc 4s, V chunk j by PV j, QT span by unit start)
    is met just in time
  - PSUM: 4 banks logits double-buffer, 2 banks PV accumulators (V|1
    ones-column trick gives numerator+denominator in one pass), 2 banks
    rotating filler accumulators
  - per-unit normalize: one [65,1024] copy pair, one sum-row DMA, one
    reciprocal, one gpsimd partition-broadcast; head0 multiplies straight
    into OT (partitions aligned), head1 goes via a bf16 staging tile + DMA
  - output y is bf16 (halves the output DMA; host sums partials in fp32)
"""

import numpy as np
import ml_dtypes

BF16 = ml_dtypes.bfloat16

P = 128
S = 2048
E = 1024
GE = 256          # embed columns per core (4 heads x 64)
KC = 8            # contraction chunks of 128 over E
JC = 16           # key chunks of 128 over S
NCORES = 8

_NC = None        # cached compiled program


def _build_program():
    import concourse.tile as tile
    from concourse import bacc, mybir

    F32 = mybir.dt.float32
    BF = mybir.dt.bfloat16
    Exp = mybir.ActivationFunctionType.Exp
    mult = mybir.AluOpType.mult
    add = mybir.AluOpType.add

    nc = bacc.Bacc(
        "TRN2",
        target_bir_lowering=False,
        debug=False,
        enable_asserts=False,
        num_devices=NCORES,
    )

    # x inputs pre-tiled on the host: [span, p, kc, w] = x^T[kc*128+p, span*w+j]
    d_xq = nc.dram_tensor("xqT", [4, P, KC, 512], BF, kind="ExternalInput")
    d_xk = nc.dram_tensor("xkT", [4, P, KC, 512], BF, kind="ExternalInput")
    d_xv = nc.dram_tensor("xvT", [8, P, KC, GE], BF, kind="ExternalInput")
    d_wq = nc.dram_tensor("wq", [P, KC, GE], BF, kind="ExternalInput")
    d_wk = nc.dram_tensor("wk", [P, KC, GE], BF, kind="ExternalInput")
    d_wv = nc.dram_tensor("wv", [P, KC, GE], BF, kind="ExternalInput")
    d_wo = nc.dram_tensor("wo", [P, 2, E], BF, kind="ExternalInput")
    d_bq = nc.dram_tensor("bqs", [P, 2], F32, kind="ExternalInput")
    d_bk = nc.dram_tensor("bks", [P, 2], F32, kind="ExternalInput")
    d_bv = nc.dram_tensor("bvb", [P, 4, 64], F32, kind="ExternalInput")
    d_bo = nc.dram_tensor("bob", [P, E], F32, kind="ExternalInput")
    d_y = nc.dram_tensor("y", [S, E], BF, kind="ExternalOutput")

    with tile.TileContext(nc) as tc:
        with (
            tc.tile_pool(name="w", bufs=1) as wpool,
            tc.tile_pool(name="xq", bufs=4) as xqp,
            tc.tile_pool(name="xk", bufs=4) as xkp,
            tc.tile_pool(name="xv", bufs=4) as xvp,
            tc.tile_pool(name="persist", bufs=1) as pers,
            tc.tile_pool(name="pt", bufs=10) as ptp,
            tc.tile_pool(name="sm", bufs=2) as sm,
            tc.tile_pool(name="y", bufs=2) as yp,
            tc.tile_pool(name="psA", bufs=2, space="PSUM") as psA,
            tc.tile_pool(name="psO", bufs=2, space="PSUM") as psO,
            tc.tile_pool(name="psF", bufs=2, space="PSUM") as psF,
        ):
            # ---- weights / biases resident in SBUF ----
            wq_t = wpool.tile([P, KC, GE], BF, tag="wq")
            wk_t = wpool.tile([P, KC, GE], BF, tag="wk")
            wv_t = wpool.tile([P, KC, GE], BF, tag="wv")
            wo_t = wpool.tile([P, 2, E], BF, tag="wo")
            bq_t = wpool.tile([P, 2], F32, tag="bq")
            bk_t = wpool.tile([P, 2], F32, tag="bk")
            bv_t = wpool.tile([P, 4, 64], F32, tag="bv")
            bo_t = wpool.tile([P, E], F32, tag="bo")

            # ---- persistent activations ----
            QT = pers.tile([P, 2, S], BF, tag="QT")       # [d'(2x128), S]
            KT = pers.tile([P, 2, S], BF, tag="KT")
            V1 = pers.tile([P, JC, 4, 65], BF, tag="V1")  # [S, (V_h|1)*4]
            OT = pers.tile([P, 2, S], BF, tag="OT")

            # ones columns (col 64 of each head block)
            nc.vector.memset(V1[:, :, :, 64:65], 1.0)
            # ones row on partition 64 for the K=1 denominator broadcast MM
            on64 = wpool.tile([P, 64], F32, tag="on64")
            nc.vector.memset(on64[:], 1.0)
            # warm-up fodder: keeps the PE HAM un-throttled while the first
            # input DMAs stream in, so the projections run at 2.4 GHz
            dum = wpool.tile([P, 512], BF, tag="dum")
            nc.vector.memset(dum[:], 0.0)

            # resident x-span slices (loaded once, shared by both c groups)
            xqs = [xqp.tile([P, KC, 512], BF, tag="xq", name=f"xqs{s}")
                   for s in range(4)]
            xks = [xkp.tile([P, KC, 512], BF, tag="xk", name=f"xks{s}")
                   for s in range(4)]
            xvs = [xvp.tile([P, KC, GE], BF, tag="xv", name=f"xvs{s}")
                   for s in range(8)]

            # ---- one FIFO DMA queue: issue order == arrival order. ----
            # Priority: prologue set (wk,xk0,wq,xq0), then exactly the order
            # the unit-0 fillers consume, then everything late-needed.
            # single FIFO DMA queue: issue order == arrival order, matching
            # consumption; first-span halves pipeline into the projections;
            # wo/bo (needed ~100us in) go last
            dma_order = [
                (wk_t, d_wk), (bk_t, d_bk),
                (xks[0][:, 0:4, :], d_xk[0, :, 0:4, :]),
                (xks[0][:, 4:8, :], d_xk[0, :, 4:8, :]),
                (wq_t, d_wq), (bq_t, d_bq),
                (xqs[0][:, 0:4, :], d_xq[0, :, 0:4, :]),
                (xqs[0][:, 4:8, :], d_xq[0, :, 4:8, :]),
                (wv_t, d_wv), (bv_t, d_bv),
                (xks[1], d_xk[1]), (xvs[0], d_xv[0]), (xvs[1], d_xv[1]),
                (xks[2], d_xk[2]), (xvs[2], d_xv[2]), (xvs[3], d_xv[3]),
                (xks[3], d_xk[3]), (xvs[4], d_xv[4]), (xvs[5], d_xv[5]),
                (xqs[1], d_xq[1]), (xvs[6], d_xv[6]), (xvs[7], d_xv[7]),
                (xqs[2], d_xq[2]), (xqs[3], d_xq[3]),
                (wo_t, d_wo), (bo_t, d_bo),
            ]
            for t, d in dma_order:
                nc.sync.dma_start(t[:], d[:])

            # ~4us of dummy matmuls to hold the PE clock at 8/8 while the
            # first input slices arrive (HAM warm-up)
            for i in range(16):
                wps = psF.tile([P, 512], F32, tag="acc", name=f"warm{i}")
                nc.tensor.matmul(wps[:], lhsT=dum[:, 0:128], rhs=dum[:],
                                 start=True, stop=True)

            # ---- filler step factories ----
            def qk_steps(w_t, b_t, dst, xs, c, sp):
                # one [128,512] span of a Q/K projection column-group as 4
                # steps of 2 accumulating MMs; bias-add copyback on the last.
                st = {}

                def step(i):
                    if i == 0:
                        st["ps"] = psF.tile([P, 512], F32, tag="acc",
                                            name=f"qkps{c}_{sp}")
                    for kc in (2 * i, 2 * i + 1):
                        nc.tensor.matmul(
                            st["ps"][:],
                            lhsT=w_t[:, kc, c * P:(c + 1) * P],
                            rhs=xs[:, kc, :],
                            start=(kc == 0), stop=(kc == KC - 1),
                        )
                    if i == 3:
                        nc.vector.tensor_scalar_add(
                            dst[:, c, sp * 512:(sp + 1) * 512], st["ps"][:],
                            b_t[:, c:c + 1])

                return [lambda i=i: step(i) for i in range(4)]

            def v_chunk(sc):
                # V projection for one 128-seq chunk: 8 accumulating MMs
                # (N=256, all 4 heads) + one strided bias-add copyback into
                # the interleaved (V_h|1)*4 layout.
                def f():
                    ps = psF.tile([P, 8, 64], F32, tag="acc",
                                  name=f"vps{sc}")
                    sg, i2 = sc // 2, sc % 2
                    for kc in range(KC):
                        nc.tensor.matmul(
                            ps[:, 0:4, :],
                            lhsT=xvs[sg][:, kc, i2 * P:(i2 + 1) * P],
                            rhs=wv_t[:, kc, :],
                            start=(kc == 0), stop=(kc == KC - 1),
                        )
                    nc.vector.tensor_tensor(
                        V1[:, sc, :, 0:64], ps[:, 0:4, :], bv_t[:], add)
                return f

            ysbs = {}

            def out_step(sc, nt, use_psA=False):
                # half of the partial out-projection for seq rows sc*128..:
                # 2 accumulating MMs (contraction over this core's 256 attn
                # dims) + bias add; DMA the row-chunk out after nt==1.
                def f():
                    if sc not in ysbs:
                        ysbs[sc] = yp.tile([P, E], BF, tag="ysb",
                                           name=f"ysb{sc}")
                    if use_psA:
                        ps = psA.tile([P, 1024], F32, tag="big",
                                      name=f"ops{sc}_{nt}")[:, 0:512]
                    else:
                        ps = psF.tile([P, 512], F32, tag="acc",
                                      name=f"ops{sc}_{nt}")
                    for cc in range(2):
                        nc.tensor.matmul(
                            ps[:],
                            lhsT=OT[:, cc, sc * P:(sc + 1) * P],
                            rhs=wo_t[:, cc, nt * 512:(nt + 1) * 512],
                            start=(cc == 0), stop=(cc == 1),
                        )
                    nc.vector.tensor_tensor(
                        ysbs[sc][:, nt * 512:(nt + 1) * 512], ps[:],
                        bo_t[:, nt * 512:(nt + 1) * 512], add)
                    if nt == 1:
                        nc.gpsimd.dma_start(
                            d_y[sc * P:(sc + 1) * P, :], ysbs.pop(sc)[:])
                return f

            # ---- prologue: K(c0,span0) + Q(c0,span0) only ----
            for f in qk_steps(wk_t, bk_t, KT, xks[0], 0, 0):
                f()
            for f in qk_steps(wq_t, bq_t, QT, xqs[0], 0, 0):
                f()

            # ---- attention units: 8 batches of 2 jc; trailing PVs and the
            # normalize of each unit are carried into the NEXT unit's first
            # batch so the exp stream never gaps at unit boundaries ----
            pend = []          # [(j, pt, c, pO0, pO1), ...] cross-unit
            carry = [None]     # pending normalize closure

            def emit_pv(j, pt, c, pO0, pO1):
                for hp, pO in ((0, pO0), (1, pO1)):
                    nc.tensor.matmul(
                        pO[:], lhsT=V1[:, j, 2 * c + hp, :],
                        rhs=pt[:, hp * 512:(hp + 1) * 512],
                        start=(j == 0), stop=(j == JC - 1),
                    )

            def normalize(c, t, pO0, pO1, mid=None, pe_bcast=False):
                # OT_h = pO[0:64] / pO[64] (row 64 = sum of P). Mid-stream
                # units broadcast 1/S via GpSimd (off the in-order PE FIFO);
                # the tail uses a K=1 PE outer product (PE is idle there).
                def f():
                    tsl = slice(t * 512, (t + 1) * 512)
                    osb = sm.tile([P, 1024], F32, tag="osb")
                    nc.vector.tensor_copy(osb[0:65, 0:512], pO0[:])
                    nc.vector.tensor_copy(osb[0:65, 512:1024], pO1[:])
                    if mid is not None:
                        mid()
                    rec = sm.tile([1, 1024], F32, tag="rec")
                    nc.sync.dma_start(rec[:], osb[64:65, :])
                    rin = sm.tile([1, 1024], F32, tag="rin")
                    nc.vector.reciprocal_approx_fast(rin[:], rec[:])
                    if pe_bcast:
                        rbs0 = psF.tile([64, 512], F32, tag="acc",
                                        name=f"rbs0_{c}{t}")
                        rbs1 = psF.tile([64, 512], F32, tag="acc",
                                        name=f"rbs1_{c}{t}")
                        nc.tensor.matmul(rbs0[:], lhsT=on64[0:1, :],
                                         rhs=rin[:, 0:512],
                                         start=True, stop=True)
                        nc.tensor.matmul(rbs1[:], lhsT=on64[0:1, :],
                                         rhs=rin[:, 512:1024],
                                         start=True, stop=True)
                        r0, r1 = rbs0[:], rbs1[:]
                    else:
                        rbs = sm.tile([64, 1024], F32, tag="rbs")
                        nc.gpsimd.partition_broadcast(rbs[:], rin[:])
                        r0, r1 = rbs[:, 0:512], rbs[:, 512:1024]
                    # head0: partitions align -> multiply straight into OT
                    nc.vector.tensor_tensor(
                        OT[0:64, c, tsl], osb[0:64, 0:512], r0, mult)
                    # head1: stage bf16 then partition-shift DMA
                    ott = sm.tile([64, 512], BF, tag="ott")
                    nc.vector.tensor_tensor(
                        ott[:], osb[0:64, 512:1024], r1, mult)
                    nc.sync.dma_start(OT[64:128, c, tsl], ott[:])
                return f

            def attn_unit(c, t, fillers):
                # fillers: list of 8 lists of callables (one per batch)
                tsl = slice(t * 512, (t + 1) * 512)
                pO0 = psO.tile([65, 512], F32, tag="pO", name=f"pO0_{c}{t}")
                pO1 = psO.tile([65, 512], F32, tag="pO", name=f"pO1_{c}{t}")

                for b in range(8):
                    for j in (2 * b, 2 * b + 1):
                        jsl = slice(j * P, (j + 1) * P)
                        pL = psA.tile([P, 1024], F32, tag="big",
                                      name=f"pL{c}{t}_{j}")
                        nc.tensor.matmul(
                            pL[:, 0:512],
                            lhsT=KT[0:64, c, jsl], rhs=QT[0:64, c, tsl],
                            start=True, stop=True,
                        )
                        nc.tensor.matmul(
                            pL[:, 512:1024],
                            lhsT=KT[64:128, c, jsl], rhs=QT[64:128, c, tsl],
                            start=True, stop=True,
                        )
                        pt = ptp.tile([P, 1024], BF, tag="pt")
                        nc.scalar.activation(pt[:], pL[:], Exp)
                        pend.append((j, pt, c, pO0, pO1))
                    while len(pend) > 2:
                        emit_pv(*pend.pop(0))
                    if b == 0 and carry[0] is not None:
                        normalize(*carry[0])()
                        carry[0] = None
                    for f in fillers[b]:
                        f()
                carry[0] = (c, t, pO0, pO1)

            # unit 0 = (0,0): V chunks (2/batch) + K(c0,s1-3) + Q(c0,s1)
            u0 = [[] for _ in range(8)]
            for b in range(8):
                u0[b] += [v_chunk(2 * b), v_chunk(2 * b + 1)]
            for b, (kind, sp) in ((0, ('K', 1)), (2, ('K', 2)),
                                  (4, ('K', 3)), (6, ('Q', 1))):
                w, bb, dst, xs = ((wk_t, bk_t, KT, xks[sp]) if kind == 'K'
                                  else (wq_t, bq_t, QT, xqs[sp]))
                steps = qk_steps(w, bb, dst, xs, 0, sp)
                u0[b] = steps[:4] + u0[b]

            # units 1-4: remaining projections, 2 steps per batch
            def spread(unit, b0, kind, c, sp):
                w, bb, dst, xs = ((wk_t, bk_t, KT, xks[sp]) if kind == 'K'
                                  else (wq_t, bq_t, QT, xqs[sp]))
                steps = qk_steps(w, bb, dst, xs, c, sp)
                unit[b0] += steps[0:2]
                unit[b0 + 1] += steps[2:4]

            u1 = [[] for _ in range(8)]
            spread(u1, 0, 'K', 1, 0)
            spread(u1, 2, 'K', 1, 1)
            spread(u1, 4, 'Q', 0, 2)
            u2 = [[] for _ in range(8)]
            spread(u2, 0, 'K', 1, 2)
            spread(u2, 2, 'K', 1, 3)
            spread(u2, 4, 'Q', 0, 3)
            spread(u2, 6, 'Q', 1, 0)
            u3 = [[] for _ in range(8)]
            spread(u3, 0, 'Q', 1, 1)
            spread(u3, 2, 'Q', 1, 2)
            u4 = [[] for _ in range(8)]
            spread(u4, 0, 'Q', 1, 3)

            def outfill(t):
                # 8 steps placed in batches 1-6 so the boundary batches
                # (0 and 7, which carry the normalize and trailing PVs)
                # stay light on the PE and DVE
                s = [out_step(sc, nt)
                     for sc in range(4 * t, 4 * t + 4) for nt in range(2)]
                return [[], [s[0], s[1]], [s[2]], [s[3]],
                        [s[4], s[5]], [s[6]], [s[7]], []]

            attn_unit(0, 0, u0)
            attn_unit(0, 1, u1)
            attn_unit(0, 2, u2)
            attn_unit(0, 3, u3)
            attn_unit(1, 0, u4)
            attn_unit(1, 1, outfill(0))
            attn_unit(1, 2, outfill(1))
            attn_unit(1, 3, outfill(2))
            # epilogue: flush the carried PVs, then run the last unit's
            # normalize with the final out-projection split in two phases:
            # the cc=0 half (depends only on long-finished c=0 OT) starts
            # right after the pO copies, on all 8 freed PSUM banks, hiding
            # under the reciprocal/broadcast chain; the cc=1 half follows
            # once the last OT columns land.
            while pend:
                emit_pv(*pend.pop(0))
            tpairs = [(sc, nt) for sc in range(12, 16) for nt in range(2)]
            taccs = {}

            def cc0_mm(acc, sc, nt):
                nc.tensor.matmul(
                    acc, lhsT=OT[:, 0, sc * P:(sc + 1) * P],
                    rhs=wo_t[:, 0, nt * 512:(nt + 1) * 512],
                    start=True, stop=False,
                )

            # phase A1: the c=0 contraction halves (only depend on OT c=0,
            # finished three units ago) start immediately after the trailing
            # PVs on the freed psA banks — keeps the PE warm and busy while
            # the normalize chain (DVE/DMA) computes 1/S
            big0 = psA.tile([P, 1024], F32, tag="big", name="ta0")
            big1 = psA.tile([P, 1024], F32, tag="big", name="ta1")
            for i, (sc, nt) in enumerate(tpairs[:4]):
                acc = [big0[:, 0:512], big0[:, 512:1024],
                       big1[:, 0:512], big1[:, 512:1024]][i]
                taccs[(sc, nt)] = acc
                cc0_mm(acc, sc, nt)

            def tail_mid():
                # phase A2: two more accs on the pO banks freed by the copies
                for i, (sc, nt) in enumerate(tpairs[4:6]):
                    acc = psO.tile([P, 512], F32, tag="pO",
                                   name=f"to{i}")[:]
                    taccs[(sc, nt)] = acc
                    cc0_mm(acc, sc, nt)

            normalize(*carry[0], mid=tail_mid, pe_bcast=True)()
            carry[0] = None
            # phase B: psF-pair cc0 (after the rbs multiplies release the
            # slots), then the c=1 halves as the last OT columns land
            for i, (sc, nt) in enumerate(tpairs[6:]):
                acc = psF.tile([P, 512], F32, tag="acc", name=f"tf{i}")[:]
                taccs[(sc, nt)] = acc
                cc0_mm(acc, sc, nt)
            for sc, nt in tpairs:
                acc = taccs[(sc, nt)]
                nc.tensor.matmul(
                    acc, lhsT=OT[:, 1, sc * P:(sc + 1) * P],
                    rhs=wo_t[:, 1, nt * 512:(nt + 1) * 512],
                    start=False, stop=True,
                )
                if sc not in ysbs:
                    ysbs[sc] = yp.tile([P, E], BF, tag="ysb",
                                       name=f"ysbt{sc}")
                nc.vector.tensor_tensor(
                    ysbs[sc][:, nt * 512:(nt + 1) * 512], acc,
                    bo_t[:, nt * 512:(nt + 1) * 512], add)
                if nt == 1:
                    nc.gpsimd.dma_start(
                        d_y[sc * P:(sc + 1) * P, :], ysbs.pop(sc)[:])

    nc.compile()
    return nc


def _get_program():
    global _NC
    if _NC is None:
        _NC = _build_program()
    return _NC


def kernel(q, k, v, Wq, bq, Wk, bk, Wv, bv, Wo, bo):
    from concourse.bass_utils import run_bass_kernel_spmd

    q = np.asarray(q, np.float32)
    k = np.asarray(k, np.float32)
    v = np.asarray(v, np.float32)
    Wq = np.asarray(Wq, np.float32)
    Wk = np.asarray(Wk, np.float32)
    Wv = np.asarray(Wv, np.float32)
    Wo = np.asarray(Wo, np.float32)
    bq = np.asarray(bq, np.float32)
    bk = np.asarray(bk, np.float32)
    bv = np.asarray(bv, np.float32)
    bo = np.asarray(bo, np.float32)

    nc = _get_program()

    def tile_qk(xb):
        # [S, E] -> x^T tiled [4, 128, KC, 512]
        return np.ascontiguousarray(
            xb.T.reshape(KC, P, 4, 512).transpose(2, 1, 0, 3)).astype(BF16)

    def tile_v(xb):
        # [S, E] -> x^T tiled [8, 128, KC, 256]
        return np.ascontiguousarray(
            xb.T.reshape(KC, P, 8, GE).transpose(2, 1, 0, 3)).astype(BF16)

    xT = {"xqT": [tile_qk(q[b]) for b in range(2)],
          "xkT": [tile_qk(k[b]) for b in range(2)],
          "xvT": [tile_v(v[b]) for b in range(2)]}

    def wprep(W, scale=1.0):
        # [E, GE] slice -> [P, KC, GE] partition-major
        return [
            np.ascontiguousarray(
                (W[:, g * GE:(g + 1) * GE] * scale)
                .reshape(KC, P, GE).transpose(1, 0, 2)
            ).astype(BF16)
            for g in range(4)
        ]

    wq_g = wprep(Wq, 0.125)
    wk_g = wprep(Wk)
    wv_g = wprep(Wv)
    wo_g = [
        np.ascontiguousarray(
            Wo[g * GE:(g + 1) * GE, :].reshape(2, P, E).transpose(1, 0, 2)
        ).astype(BF16)
        for g in range(4)
    ]
    bq_g = [np.ascontiguousarray((bq[g * GE:(g + 1) * GE] * 0.125)
                                 .reshape(2, P).T).astype(np.float32)
            for g in range(4)]
    bk_g = [np.ascontiguousarray(bk[g * GE:(g + 1) * GE].reshape(2, P).T)
            .astype(np.float32) for g in range(4)]
    bv_g = [np.ascontiguousarray(np.broadcast_to(
        bv[g * GE:(g + 1) * GE].astype(np.float32),
        (P, GE))).reshape(P, 4, 64) for g in range(4)]
    bo_full = np.ascontiguousarray(
        np.broadcast_to(bo.astype(np.float32), (P, E)))
    bo_zero = np.zeros((P, E), np.float32)

    in_maps = []
    for c in range(NCORES):
        b, g = divmod(c, 4)
        in_maps.append({
            "xqT": xT["xqT"][b],
            "xkT": xT["xkT"][b],
            "xvT": xT["xvT"][b],
            "wq": wq_g[g], "wk": wk_g[g], "wv": wv_g[g], "wo": wo_g[g],
            "bqs": bq_g[g], "bks": bk_g[g], "bvb": bv_g[g],
            "bob": bo_full if g == 0 else bo_zero,
        })

    res = run_bass_kernel_spmd(nc, in_maps, list(range(NCORES)),
                               **_RUN_KWARGS)
    globals()["LAST_RESULTS"] = res

    parts = [np.asarray(res.results[c]["y"], np.float32)
             for c in range(NCORES)]
    out = np.stack([
        parts[0] + parts[1] + parts[2] + parts[3],
        parts[4] + parts[5] + parts[6] + parts[7],
    ]).astype(np.float32)
    return out


# test-harness hooks (kernel.py itself never enables tracing)
_RUN_KWARGS = {}
LAST_RESULTS = None


# revision 27
# speedup vs baseline: 1.0079x; 1.0079x over previous
"""Multi-head self-attention (B=2, S=2048, E=1024, H=16) on 8 Trainium2 cores.

Sharding: 2D (batch x head-group). Core c handles batch b = c // 4 and head
group g = c % 4 (4 heads, 256 embed columns). Each core computes its QKV
projection slices, fused attention for its 4 heads, and a partial output
projection (attn_g @ Wo[g_slice]); the host sums the 4 partials per batch
(the head-concat contraction) and stacks the 2 batches.

Device structure (v2 — ScalarE-exp / TensorE co-bound pipeline):
  - all input x spans are loaded ONCE (each [128,KC,span] slice is shared by
    the c=0 and c=1 column groups of its projection) through the single
    hardware DMA FIFO in consumption-priority order, so the first logits
    fire ~3MB into the stream instead of after the whole input set
  - attention runs in 2-jc batches: [4 row-packed K=64 logits MMs]
    [2 exps] [4 PV MMs lagging 2 jc] [fillers]; the PV lag keeps the PE
    FIFO from ever waiting on ScalarE, and batching halves the PE
    64-row/128-row tiling-mode switches
  - projections (K/Q column groups, V chunks) and the partial output
    projection ride as fillers in the batch slots, scheduled so every
    dependency (KT span by jc 4s, V chunk j by PV j, QT span by unit start)
    is met just in time
  - PSUM: 4 banks logits double-buffer, 2 banks PV accumulators (V|1
    ones-column trick gives numerator+denominator in one pass), 2 banks
    rotating filler accumulators
  - per-unit normalize: one [65,1024] copy pair, one sum-row DMA, one
    reciprocal, one gpsimd partition-broadcast; head0 multiplies straight
    into OT (partitions aligned), head1 goes via a bf16 staging tile + DMA
  - output y is bf16 (halves the output DMA; host sums partials in fp32)
"""

import numpy as np
import ml_dtypes

BF16 = ml_dtypes.bfloat16

P = 128
S = 2048
E = 1024
GE = 256          # embed columns per core (4 heads x 64)
KC = 8            # contraction chunks of 128 over E
JC = 16           # key chunks of 128 over S
NCORES = 8

_NC = None        # cached compiled program


def _build_program():
    import concourse.tile as tile
    from concourse import bacc, mybir

    F32 = mybir.dt.float32
    BF = mybir.dt.bfloat16
    Exp = mybir.ActivationFunctionType.Exp
    mult = mybir.AluOpType.mult
    add = mybir.AluOpType.add

    nc = bacc.Bacc(
        "TRN2",
        target_bir_lowering=False,
        debug=False,
        enable_asserts=False,
        num_devices=NCORES,
    )

    # x inputs pre-tiled on the host: [span, p, kc, w] = x^T[kc*128+p, span*w+j]
    d_xq = nc.dram_tensor("xqT", [4, P, KC, 512], BF, kind="ExternalInput")
    d_xk = nc.dram_tensor("xkT", [4, P, KC, 512], BF, kind="ExternalInput")
    d_xv = nc.dram_tensor("xvT", [8, P, KC, GE], BF, kind="ExternalInput")
    d_wq = nc.dram_tensor("wq", [P, KC, GE], BF, kind="ExternalInput")
    d_wk = nc.dram_tensor("wk", [P, KC, GE], BF, kind="ExternalInput")
    d_wv = nc.dram_tensor("wv", [P, KC, GE], BF, kind="ExternalInput")
    d_wo = nc.dram_tensor("wo", [P, 2, E], BF, kind="ExternalInput")
    d_bq = nc.dram_tensor("bqs", [P, 2], F32, kind="ExternalInput")
    d_bk = nc.dram_tensor("bks", [P, 2], F32, kind="ExternalInput")
    d_bv = nc.dram_tensor("bvb", [P, 4, 64], F32, kind="ExternalInput")
    d_bo = nc.dram_tensor("bob", [P, E], F32, kind="ExternalInput")
    d_y = nc.dram_tensor("y", [S, E], BF, kind="ExternalOutput")

    with tile.TileContext(nc) as tc:
        with (
            tc.tile_pool(name="w", bufs=1) as wpool,
            tc.tile_pool(name="xq", bufs=4) as xqp,
            tc.tile_pool(name="xk", bufs=4) as xkp,
            tc.tile_pool(name="xv", bufs=4) as xvp,
            tc.tile_pool(name="persist", bufs=1) as pers,
            tc.tile_pool(name="pt", bufs=10) as ptp,
            tc.tile_pool(name="sm", bufs=2) as sm,
            tc.tile_pool(name="y", bufs=2) as yp,
            tc.tile_pool(name="psA", bufs=2, space="PSUM") as psA,
            tc.tile_pool(name="psO", bufs=2, space="PSUM") as psO,
            tc.tile_pool(name="psF", bufs=2, space="PSUM") as psF,
        ):
            # ---- weights / biases resident in SBUF ----
            wq_t = wpool.tile([P, KC, GE], BF, tag="wq")
            wk_t = wpool.tile([P, KC, GE], BF, tag="wk")
            wv_t = wpool.tile([P, KC, GE], BF, tag="wv")
            wo_t = wpool.tile([P, 2, E], BF, tag="wo")
            bq_t = wpool.tile([P, 2], F32, tag="bq")
            bk_t = wpool.tile([P, 2], F32, tag="bk")
            bv_t = wpool.tile([P, 4, 64], F32, tag="bv")
            bo_t = wpool.tile([P, E], F32, tag="bo")

            # ---- persistent activations ----
            QT = pers.tile([P, 2, S], BF, tag="QT")       # [d'(2x128), S]
            KT = pers.tile([P, 2, S], BF, tag="KT")
            V1 = pers.tile([P, JC, 4, 65], BF, tag="V1")  # [S, (V_h|1)*4]
            OT = pers.tile([P, 2, S], BF, tag="OT")

            # ones columns (col 64 of each head block)
            nc.vector.memset(V1[:, :, :, 64:65], 1.0)
            # warm-up fodder: keeps the PE HAM un-throttled while the first
            # input DMAs stream in, so the projections run at 2.4 GHz
            dum = wpool.tile([P, 512], BF, tag="dum")
            nc.vector.memset(dum[:], 0.0)

            # resident x-span slices (loaded once, shared by both c groups)
            xqs = [xqp.tile([P, KC, 512], BF, tag="xq", name=f"xqs{s}")
                   for s in range(4)]
            xks = [xkp.tile([P, KC, 512], BF, tag="xk", name=f"xks{s}")
                   for s in range(4)]
            xvs = [xvp.tile([P, KC, GE], BF, tag="xv", name=f"xvs{s}")
                   for s in range(8)]

            # ---- one FIFO DMA queue: issue order == arrival order. ----
            # Priority: prologue set (wk,xk0,wq,xq0), then exactly the order
            # the unit-0 fillers consume, then everything late-needed.
            # single FIFO DMA queue: issue order == arrival order, matching
            # consumption; first-span halves pipeline into the projections;
            # wo/bo (needed ~100us in) go last
            dma_order = [
                (wk_t, d_wk), (bk_t, d_bk),
                (xks[0][:, 0:4, :], d_xk[0, :, 0:4, :]),
                (xks[0][:, 4:8, :], d_xk[0, :, 4:8, :]),
                (wq_t, d_wq), (bq_t, d_bq),
                (xqs[0][:, 0:4, :], d_xq[0, :, 0:4, :]),
                (xqs[0][:, 4:8, :], d_xq[0, :, 4:8, :]),
                (wv_t, d_wv), (bv_t, d_bv),
                (xks[1], d_xk[1]), (xvs[0], d_xv[0]), (xvs[1], d_xv[1]),
                (xks[2], d_xk[2]), (xvs[2], d_xv[2]), (xvs[3], d_xv[3]),
                (xks[3], d_xk[3]), (xvs[4], d_xv[4]), (xvs[5], d_xv[5]),
                (xqs[1], d_xq[1]), (xvs[6], d_xv[6]), (xvs[7], d_xv[7]),
                (xqs[2], d_xq[2]), (xqs[3], d_xq[3]),
                (wo_t, d_wo), (bo_t, d_bo),
            ]
            for t, d in dma_order:
                nc.sync.dma_start(t[:], d[:])

            # ~4us of dummy matmuls to hold the PE clock at 8/8 while the
            # first input slices arrive (HAM warm-up)
            for i in range(16):
                wps = psF.tile([P, 512], F32, tag="acc", name=f"warm{i}")
                nc.tensor.matmul(wps[:], lhsT=dum[:, 0:128], rhs=dum[:],
                                 start=True, stop=True)

            # ---- filler step factories ----
            def qk_steps(w_t, b_t, dst, xs, c, sp):
                # one [128,512] span of a Q/K projection column-group as 4
                # steps of 2 accumulating MMs; bias-add copyback on the last.
                st = {}

                def step(i):
                    if i == 0:
                        st["ps"] = psF.tile([P, 512], F32, tag="acc",
                                            name=f"qkps{c}_{sp}")
                    for kc in (2 * i, 2 * i + 1):
                        nc.tensor.matmul(
                            st["ps"][:],
                            lhsT=w_t[:, kc, c * P:(c + 1) * P],
                            rhs=xs[:, kc, :],
                            start=(kc == 0), stop=(kc == KC - 1),
                        )
                    if i == 3:
                        nc.vector.tensor_scalar_add(
                            dst[:, c, sp * 512:(sp + 1) * 512], st["ps"][:],
                            b_t[:, c:c + 1])

                return [lambda i=i: step(i) for i in range(4)]

            def v_chunk(sc):
                # V projection for one 128-seq chunk: 8 accumulating MMs
                # (N=256, all 4 heads) + one strided bias-add copyback into
                # the interleaved (V_h|1)*4 layout.
                def f():
                    ps = psF.tile([P, 8, 64], F32, tag="acc",
                                  name=f"vps{sc}")
                    sg, i2 = sc // 2, sc % 2
                    for kc in range(KC):
                        nc.tensor.matmul(
                            ps[:, 0:4, :],
                            lhsT=xvs[sg][:, kc, i2 * P:(i2 + 1) * P],
                            rhs=wv_t[:, kc, :],
                            start=(kc == 0), stop=(kc == KC - 1),
                        )
                    nc.vector.tensor_tensor(
                        V1[:, sc, :, 0:64], ps[:, 0:4, :], bv_t[:], add)
                return f

            ysbs = {}

            def out_step(sc, nt, use_psA=False):
                # half of the partial out-projection for seq rows sc*128..:
                # 2 accumulating MMs (contraction over this core's 256 attn
                # dims) + bias add; DMA the row-chunk out after nt==1.
                def f():
                    if sc not in ysbs:
                        ysbs[sc] = yp.tile([P, E], BF, tag="ysb",
                                           name=f"ysb{sc}")
                    if use_psA:
                        ps = psA.tile([P, 1024], F32, tag="big",
                                      name=f"ops{sc}_{nt}")[:, 0:512]
                    else:
                        ps = psF.tile([P, 512], F32, tag="acc",
                                      name=f"ops{sc}_{nt}")
                    for cc in range(2):
                        nc.tensor.matmul(
                            ps[:],
                            lhsT=OT[:, cc, sc * P:(sc + 1) * P],
                            rhs=wo_t[:, cc, nt * 512:(nt + 1) * 512],
                            start=(cc == 0), stop=(cc == 1),
                        )
                    nc.vector.tensor_tensor(
                        ysbs[sc][:, nt * 512:(nt + 1) * 512], ps[:],
                        bo_t[:, nt * 512:(nt + 1) * 512], add)
                    if nt == 1:
                        nc.gpsimd.dma_start(
                            d_y[sc * P:(sc + 1) * P, :], ysbs.pop(sc)[:])
                return f

            # ---- prologue: K(c0,span0) + Q(c0,span0) only ----
            for f in qk_steps(wk_t, bk_t, KT, xks[0], 0, 0):
                f()
            for f in qk_steps(wq_t, bq_t, QT, xqs[0], 0, 0):
                f()

            # ---- attention units: 8 batches of 2 jc; trailing PVs and the
            # normalize of each unit are carried into the NEXT unit's first
            # batch so the exp stream never gaps at unit boundaries ----
            pend = []          # [(j, pt, c, pO0, pO1), ...] cross-unit
            carry = [None]     # pending normalize closure

            def emit_pv(j, pt, c, pO0, pO1):
                for hp, pO in ((0, pO0), (1, pO1)):
                    nc.tensor.matmul(
                        pO[:], lhsT=V1[:, j, 2 * c + hp, :],
                        rhs=pt[:, hp * 512:(hp + 1) * 512],
                        start=(j == 0), stop=(j == JC - 1),
                    )

            def normalize(c, t, pO0, pO1, mid=None):
                # OT_h = pO[0:64] / pO[64] (row 64 = sum of P). 1/S is
                # broadcast across partitions by GpSimd, off the PE FIFO.
                def f():
                    tsl = slice(t * 512, (t + 1) * 512)
                    osb = sm.tile([P, 1024], F32, tag="osb")
                    nc.vector.tensor_copy(osb[0:65, 0:512], pO0[:])
                    nc.vector.tensor_copy(osb[0:65, 512:1024], pO1[:])
                    if mid is not None:
                        mid()
                    rec = sm.tile([1, 1024], F32, tag="rec")
                    nc.sync.dma_start(rec[:], osb[64:65, :])
                    rin = sm.tile([1, 1024], F32, tag="rin")
                    nc.vector.reciprocal_approx_fast(rin[:], rec[:])
                    rbs = sm.tile([64, 1024], F32, tag="rbs")
                    nc.gpsimd.partition_broadcast(rbs[:], rin[:])
                    r0, r1 = rbs[:, 0:512], rbs[:, 512:1024]
                    # head0: partitions align -> multiply straight into OT
                    nc.vector.tensor_tensor(
                        OT[0:64, c, tsl], osb[0:64, 0:512], r0, mult)
                    # head1: stage bf16 then partition-shift DMA
                    ott = sm.tile([64, 512], BF, tag="ott")
                    nc.vector.tensor_tensor(
                        ott[:], osb[0:64, 512:1024], r1, mult)
                    nc.sync.dma_start(OT[64:128, c, tsl], ott[:])
                return f

            def attn_unit(c, t, fillers):
                # fillers: list of 8 lists of callables (one per batch)
                tsl = slice(t * 512, (t + 1) * 512)
                pO0 = psO.tile([65, 512], F32, tag="pO", name=f"pO0_{c}{t}")
                pO1 = psO.tile([65, 512], F32, tag="pO", name=f"pO1_{c}{t}")

                for b in range(8):
                    for j in (2 * b, 2 * b + 1):
                        jsl = slice(j * P, (j + 1) * P)
                        pL = psA.tile([P, 1024], F32, tag="big",
                                      name=f"pL{c}{t}_{j}")
                        nc.tensor.matmul(
                            pL[:, 0:512],
                            lhsT=KT[0:64, c, jsl], rhs=QT[0:64, c, tsl],
                            start=True, stop=True,
                        )
                        nc.tensor.matmul(
                            pL[:, 512:1024],
                            lhsT=KT[64:128, c, jsl], rhs=QT[64:128, c, tsl],
                            start=True, stop=True,
                        )
                        pt = ptp.tile([P, 1024], BF, tag="pt")
                        nc.scalar.activation(pt[:], pL[:], Exp)
                        pend.append((j, pt, c, pO0, pO1))
                    while len(pend) > 2:
                        emit_pv(*pend.pop(0))
                    if b == 0 and carry[0] is not None:
                        normalize(*carry[0])()
                        carry[0] = None
                    for f in fillers[b]:
                        f()
                carry[0] = (c, t, pO0, pO1)

            # unit 0 = (0,0): V chunks (2/batch) + K(c0,s1-3) + Q(c0,s1)
            u0 = [[] for _ in range(8)]
            for b in range(8):
                u0[b] += [v_chunk(2 * b), v_chunk(2 * b + 1)]
            for b, (kind, sp) in ((0, ('K', 1)), (2, ('K', 2)),
                                  (4, ('K', 3)), (6, ('Q', 1))):
                w, bb, dst, xs = ((wk_t, bk_t, KT, xks[sp]) if kind == 'K'
                                  else (wq_t, bq_t, QT, xqs[sp]))
                steps = qk_steps(w, bb, dst, xs, 0, sp)
                u0[b] = steps[:4] + u0[b]

            # units 1-4: remaining projections, 2 steps per batch
            def spread(unit, b0, kind, c, sp):
                w, bb, dst, xs = ((wk_t, bk_t, KT, xks[sp]) if kind == 'K'
                                  else (wq_t, bq_t, QT, xqs[sp]))
                steps = qk_steps(w, bb, dst, xs, c, sp)
                unit[b0] += steps[0:2]
                unit[b0 + 1] += steps[2:4]

            u1 = [[] for _ in range(8)]
            spread(u1, 0, 'K', 1, 0)
            spread(u1, 2, 'K', 1, 1)
            spread(u1, 4, 'Q', 0, 2)
            u2 = [[] for _ in range(8)]
            spread(u2, 0, 'K', 1, 2)
            spread(u2, 2, 'K', 1, 3)
            spread(u2, 4, 'Q', 0, 3)
            spread(u2, 6, 'Q', 1, 0)
            u3 = [[] for _ in range(8)]
            spread(u3, 0, 'Q', 1, 1)
            spread(u3, 2, 'Q', 1, 2)
            u4 = [[] for _ in range(8)]
            spread(u4, 0, 'Q', 1, 3)

            def outfill(t):
                # 8 steps placed in batches 1-6 so the boundary batches
                # (0 and 7, which carry the normalize and trailing PVs)
                # stay light on the PE and DVE
                s = [out_step(sc, nt)
                     for sc in range(4 * t, 4 * t + 4) for nt in range(2)]
                return [[], [s[0], s[1]], [s[2]], [s[3]],
                        [s[4], s[5]], [s[6]], [s[7]], []]

            attn_unit(0, 0, u0)
            attn_unit(0, 1, u1)
            attn_unit(0, 2, u2)
            attn_unit(0, 3, u3)
            attn_unit(1, 0, u4)
            attn_unit(1, 1, outfill(0))
            attn_unit(1, 2, outfill(1))
            attn_unit(1, 3, outfill(2))
            # epilogue: flush the carried PVs, then run the last unit's
            # normalize with the final out-projection split in two phases:
            # the cc=0 half (depends only on long-finished c=0 OT) starts
            # right after the pO copies, on all 8 freed PSUM banks, hiding
            # under the reciprocal/broadcast chain; the cc=1 half follows
            # once the last OT columns land.
            while pend:
                emit_pv(*pend.pop(0))
            tpairs = [(sc, nt) for sc in range(12, 16) for nt in range(2)]
            taccs = {}

            def cc0_mm(acc, sc, nt):
                nc.tensor.matmul(
                    acc, lhsT=OT[:, 0, sc * P:(sc + 1) * P],
                    rhs=wo_t[:, 0, nt * 512:(nt + 1) * 512],
                    start=True, stop=False,
                )

            # phase A1: the c=0 contraction halves (only depend on OT c=0,
            # finished three units ago) start immediately after the trailing
            # PVs on the freed psA banks — keeps the PE warm and busy while
            # the normalize chain (DVE/DMA) computes 1/S
            big0 = psA.tile([P, 1024], F32, tag="big", name="ta0")
            big1 = psA.tile([P, 1024], F32, tag="big", name="ta1")
            for i, (sc, nt) in enumerate(tpairs[:4]):
                acc = [big0[:, 0:512], big0[:, 512:1024],
                       big1[:, 0:512], big1[:, 512:1024]][i]
                taccs[(sc, nt)] = acc
                cc0_mm(acc, sc, nt)

            for i, (sc, nt) in enumerate(tpairs[6:]):
                acc = psF.tile([P, 512], F32, tag="acc", name=f"tf{i}")[:]
                taccs[(sc, nt)] = acc
                cc0_mm(acc, sc, nt)

            def tail_mid():
                # phase A2: two more accs on the pO banks freed by the copies
                for i, (sc, nt) in enumerate(tpairs[4:6]):
                    acc = psO.tile([P, 512], F32, tag="pO",
                                   name=f"to{i}")[:]
                    taccs[(sc, nt)] = acc
                    cc0_mm(acc, sc, nt)

            normalize(*carry[0], mid=tail_mid)()
            carry[0] = None
            # phase B: the c=1 halves as the last OT columns land
            for sc, nt in tpairs:
                acc = taccs[(sc, nt)]
                nc.tensor.matmul(
                    acc, lhsT=OT[:, 1, sc * P:(sc + 1) * P],
                    rhs=wo_t[:, 1, nt * 512:(nt + 1) * 512],
                    start=False, stop=True,
                )
                if sc not in ysbs:
                    ysbs[sc] = yp.tile([P, E], BF, tag="ysb",
                                       name=f"ysbt{sc}")
                nc.vector.tensor_tensor(
                    ysbs[sc][:, nt * 512:(nt + 1) * 512], acc,
                    bo_t[:, nt * 512:(nt + 1) * 512], add)
                if nt == 1:
                    nc.gpsimd.dma_start(
                        d_y[sc * P:(sc + 1) * P, :], ysbs.pop(sc)[:])

    nc.compile()
    return nc


def _get_program():
    global _NC
    if _NC is None:
        _NC = _build_program()
    return _NC


def kernel(q, k, v, Wq, bq, Wk, bk, Wv, bv, Wo, bo):
    from concourse.bass_utils import run_bass_kernel_spmd

    q = np.asarray(q, np.float32)
    k = np.asarray(k, np.float32)
    v = np.asarray(v, np.float32)
    Wq = np.asarray(Wq, np.float32)
    Wk = np.asarray(Wk, np.float32)
    Wv = np.asarray(Wv, np.float32)
    Wo = np.asarray(Wo, np.float32)
    bq = np.asarray(bq, np.float32)
    bk = np.asarray(bk, np.float32)
    bv = np.asarray(bv, np.float32)
    bo = np.asarray(bo, np.float32)

    nc = _get_program()

    def tile_qk(xb):
        # [S, E] -> x^T tiled [4, 128, KC, 512]
        return np.ascontiguousarray(
            xb.T.reshape(KC, P, 4, 512).transpose(2, 1, 0, 3)).astype(BF16)

    def tile_v(xb):
        # [S, E] -> x^T tiled [8, 128, KC, 256]
        return np.ascontiguousarray(
            xb.T.reshape(KC, P, 8, GE).transpose(2, 1, 0, 3)).astype(BF16)

    xT = {"xqT": [tile_qk(q[b]) for b in range(2)],
          "xkT": [tile_qk(k[b]) for b in range(2)],
          "xvT": [tile_v(v[b]) for b in range(2)]}

    def wprep(W, scale=1.0):
        # [E, GE] slice -> [P, KC, GE] partition-major
        return [
            np.ascontiguousarray(
                (W[:, g * GE:(g + 1) * GE] * scale)
                .reshape(KC, P, GE).transpose(1, 0, 2)
            ).astype(BF16)
            for g in range(4)
        ]

    wq_g = wprep(Wq, 0.125)
    wk_g = wprep(Wk)
    wv_g = wprep(Wv)
    wo_g = [
        np.ascontiguousarray(
            Wo[g * GE:(g + 1) * GE, :].reshape(2, P, E).transpose(1, 0, 2)
        ).astype(BF16)
        for g in range(4)
    ]
    bq_g = [np.ascontiguousarray((bq[g * GE:(g + 1) * GE] * 0.125)
                                 .reshape(2, P).T).astype(np.float32)
            for g in range(4)]
    bk_g = [np.ascontiguousarray(bk[g * GE:(g + 1) * GE].reshape(2, P).T)
            .astype(np.float32) for g in range(4)]
    bv_g = [np.ascontiguousarray(np.broadcast_to(
        bv[g * GE:(g + 1) * GE].astype(np.float32),
        (P, GE))).reshape(P, 4, 64) for g in range(4)]
    bo_full = np.ascontiguousarray(
        np.broadcast_to(bo.astype(np.float32), (P, E)))
    bo_zero = np.zeros((P, E), np.float32)

    in_maps = []
    for c in range(NCORES):
        b, g = divmod(c, 4)
        in_maps.append({
            "xqT": xT["xqT"][b],
            "xkT": xT["xkT"][b],
            "xvT": xT["xvT"][b],
            "wq": wq_g[g], "wk": wk_g[g], "wv": wv_g[g], "wo": wo_g[g],
            "bqs": bq_g[g], "bks": bk_g[g], "bvb": bv_g[g],
            "bob": bo_full if g == 0 else bo_zero,
        })

    res = run_bass_kernel_spmd(nc, in_maps, list(range(NCORES)),
                               **_RUN_KWARGS)
    globals()["LAST_RESULTS"] = res

    parts = [np.asarray(res.results[c]["y"], np.float32)
             for c in range(NCORES)]
    out = np.stack([
        parts[0] + parts[1] + parts[2] + parts[3],
        parts[4] + parts[5] + parts[6] + parts[7],
    ]).astype(np.float32)
    return out


# test-harness hooks (kernel.py itself never enables tracing)
_RUN_KWARGS = {}
LAST_RESULTS = None


# revision 28
# speedup vs baseline: 1.0318x; 1.0237x over previous
"""Multi-head self-attention (B=2, S=2048, E=1024, H=16) on 8 Trainium2 cores.

Sharding: 2D (batch x head-group). Core c handles batch b = c // 4 and head
group g = c % 4 (4 heads, 256 embed columns). Each core computes its QKV
projection slices, fused attention for its 4 heads, and a partial output
projection (attn_g @ Wo[g_slice]); the host sums the 4 partials per batch
(the head-concat contraction) and stacks the 2 batches.

Device structure (v2 — ScalarE-exp / TensorE co-bound pipeline):
  - all input x spans are loaded ONCE (each [128,KC,span] slice is shared by
    the c=0 and c=1 column groups of its projection) through the single
    hardware DMA FIFO in consumption-priority order, so the first logits
    fire ~3MB into the stream instead of after the whole input set
  - attention runs in 2-jc batches: [4 row-packed K=64 logits MMs]
    [2 exps] [4 PV MMs lagging 2 jc] [fillers]; the PV lag keeps the PE
    FIFO from ever waiting on ScalarE, and batching halves the PE
    64-row/128-row tiling-mode switches
  - projections (K/Q column groups, V chunks) and the partial output
    projection ride as fillers in the batch slots, scheduled so every
    dependency (KT span by jc 4s, V chunk j by PV j, QT span by unit start)
    is met just in time
  - PSUM: 4 banks logits double-buffer, 2 banks PV accumulators (V|1
    ones-column trick gives numerator+denominator in one pass), 2 banks
    rotating filler accumulators
  - per-unit normalize: one [65,1024] copy pair, one sum-row DMA, one
    reciprocal, one gpsimd partition-broadcast; head0 multiplies straight
    into OT (partitions aligned), head1 goes via a bf16 staging tile + DMA
  - output y is bf16 (halves the output DMA; host sums partials in fp32)
"""

import numpy as np
import ml_dtypes

BF16 = ml_dtypes.bfloat16

P = 128
S = 2048
E = 1024
GE = 256          # embed columns per core (4 heads x 64)
KC = 8            # contraction chunks of 128 over E
JC = 16           # key chunks of 128 over S
NCORES = 8

_NC = None        # cached compiled program


def _build_program():
    import concourse.tile as tile
    from concourse import bacc, mybir

    F32 = mybir.dt.float32
    BF = mybir.dt.bfloat16
    Exp = mybir.ActivationFunctionType.Exp
    mult = mybir.AluOpType.mult
    add = mybir.AluOpType.add

    nc = bacc.Bacc(
        "TRN2",
        target_bir_lowering=False,
        debug=False,
        enable_asserts=False,
        num_devices=NCORES,
    )

    # x inputs pre-tiled on the host: [span, p, kc, w] = x^T[kc*128+p, span*w+j]
    d_xq = nc.dram_tensor("xqT", [4, P, KC, 512], BF, kind="ExternalInput")
    d_xk = nc.dram_tensor("xkT", [4, P, KC, 512], BF, kind="ExternalInput")
    d_xv = nc.dram_tensor("xvT", [8, P, KC, GE], BF, kind="ExternalInput")
    d_wq = nc.dram_tensor("wq", [P, KC, GE], BF, kind="ExternalInput")
    d_wk = nc.dram_tensor("wk", [P, KC, GE], BF, kind="ExternalInput")
    d_wv = nc.dram_tensor("wv", [P, KC, GE], BF, kind="ExternalInput")
    d_wo = nc.dram_tensor("wo", [P, 2, E], BF, kind="ExternalInput")
    d_bq = nc.dram_tensor("bqs", [P, 2], F32, kind="ExternalInput")
    d_bk = nc.dram_tensor("bks", [P, 2], F32, kind="ExternalInput")
    d_bv = nc.dram_tensor("bvb", [P, 4, 64], F32, kind="ExternalInput")
    d_bo = nc.dram_tensor("bob", [P, E], F32, kind="ExternalInput")
    d_y = nc.dram_tensor("y", [S, E], BF, kind="ExternalOutput")

    with tile.TileContext(nc) as tc:
        with (
            tc.tile_pool(name="w", bufs=1) as wpool,
            tc.tile_pool(name="xq", bufs=4) as xqp,
            tc.tile_pool(name="xk", bufs=4) as xkp,
            tc.tile_pool(name="xv", bufs=4) as xvp,
            tc.tile_pool(name="persist", bufs=1) as pers,
            tc.tile_pool(name="pt", bufs=10) as ptp,
            tc.tile_pool(name="sm", bufs=2) as sm,
            tc.tile_pool(name="y", bufs=4) as yp,
            tc.tile_pool(name="psA", bufs=2, space="PSUM") as psA,
            tc.tile_pool(name="psO", bufs=2, space="PSUM") as psO,
            tc.tile_pool(name="psF", bufs=2, space="PSUM") as psF,
        ):
            # ---- weights / biases resident in SBUF ----
            wq_t = wpool.tile([P, KC, GE], BF, tag="wq")
            wk_t = wpool.tile([P, KC, GE], BF, tag="wk")
            wv_t = wpool.tile([P, KC, GE], BF, tag="wv")
            wo_t = wpool.tile([P, 2, E], BF, tag="wo")
            bq_t = wpool.tile([P, 2], F32, tag="bq")
            bk_t = wpool.tile([P, 2], F32, tag="bk")
            bv_t = wpool.tile([P, 4, 64], F32, tag="bv")
            bo_t = wpool.tile([P, E], F32, tag="bo")

            # ---- persistent activations ----
            QT = pers.tile([P, 2, S], BF, tag="QT")       # [d'(2x128), S]
            KT = pers.tile([P, 2, S], BF, tag="KT")
            V1 = pers.tile([P, JC, 4, 65], BF, tag="V1")  # [S, (V_h|1)*4]
            OT = pers.tile([P, 2, S], BF, tag="OT")

            # ones columns (col 64 of each head block)
            nc.vector.memset(V1[:, :, :, 64:65], 1.0)
            # warm-up fodder: keeps the PE HAM un-throttled while the first
            # input DMAs stream in, so the projections run at 2.4 GHz
            dum = wpool.tile([P, 512], BF, tag="dum")
            nc.vector.memset(dum[:], 0.0)

            # resident x-span slices (loaded once, shared by both c groups)
            xqs = [xqp.tile([P, KC, 512], BF, tag="xq", name=f"xqs{s}")
                   for s in range(4)]
            xks = [xkp.tile([P, KC, 512], BF, tag="xk", name=f"xks{s}")
                   for s in range(4)]
            xvs = [xvp.tile([P, KC, GE], BF, tag="xv", name=f"xvs{s}")
                   for s in range(8)]

            # ---- one FIFO DMA queue: issue order == arrival order. ----
            # Priority: prologue set (wk,xk0,wq,xq0), then exactly the order
            # the unit-0 fillers consume, then everything late-needed.
            # single FIFO DMA queue: issue order == arrival order, matching
            # consumption; first-span halves pipeline into the projections;
            # wo/bo (needed ~100us in) go last
            dma_order = [
                (wk_t, d_wk), (bk_t, d_bk),
                (xks[0][:, 0:4, :], d_xk[0, :, 0:4, :]),
                (xks[0][:, 4:8, :], d_xk[0, :, 4:8, :]),
                (wq_t, d_wq), (bq_t, d_bq),
                (xqs[0][:, 0:4, :], d_xq[0, :, 0:4, :]),
                (xqs[0][:, 4:8, :], d_xq[0, :, 4:8, :]),
                (wv_t, d_wv), (bv_t, d_bv),
                (xks[1], d_xk[1]), (xvs[0], d_xv[0]), (xvs[1], d_xv[1]),
                (xks[2], d_xk[2]), (xvs[2], d_xv[2]), (xvs[3], d_xv[3]),
                (xks[3], d_xk[3]), (xvs[4], d_xv[4]), (xvs[5], d_xv[5]),
                (xqs[1], d_xq[1]), (xvs[6], d_xv[6]), (xvs[7], d_xv[7]),
                (xqs[2], d_xq[2]), (xqs[3], d_xq[3]),
                (wo_t, d_wo), (bo_t, d_bo),
            ]
            for t, d in dma_order:
                nc.sync.dma_start(t[:], d[:])

            # ~4us of dummy matmuls to hold the PE clock at 8/8 while the
            # first input slices arrive (HAM warm-up)
            for i in range(16):
                wps = psF.tile([P, 512], F32, tag="acc", name=f"warm{i}")
                nc.tensor.matmul(wps[:], lhsT=dum[:, 0:128], rhs=dum[:],
                                 start=True, stop=True)

            # ---- filler step factories ----
            def qk_steps(w_t, b_t, dst, xs, c, sp):
                # one [128,512] span of a Q/K projection column-group as 4
                # steps of 2 accumulating MMs; bias-add copyback on the last.
                st = {}

                def step(i):
                    if i == 0:
                        st["ps"] = psF.tile([P, 512], F32, tag="acc",
                                            name=f"qkps{c}_{sp}")
                    for kc in (2 * i, 2 * i + 1):
                        nc.tensor.matmul(
                            st["ps"][:],
                            lhsT=w_t[:, kc, c * P:(c + 1) * P],
                            rhs=xs[:, kc, :],
                            start=(kc == 0), stop=(kc == KC - 1),
                        )
                    if i == 3:
                        nc.vector.tensor_scalar_add(
                            dst[:, c, sp * 512:(sp + 1) * 512], st["ps"][:],
                            b_t[:, c:c + 1])

                return [lambda i=i: step(i) for i in range(4)]

            def v_chunk(sc):
                # V projection for one 128-seq chunk: 8 accumulating MMs
                # (N=256, all 4 heads) + one strided bias-add copyback into
                # the interleaved (V_h|1)*4 layout.
                def f():
                    ps = psF.tile([P, 8, 64], F32, tag="acc",
                                  name=f"vps{sc}")
                    sg, i2 = sc // 2, sc % 2
                    for kc in range(KC):
                        nc.tensor.matmul(
                            ps[:, 0:4, :],
                            lhsT=xvs[sg][:, kc, i2 * P:(i2 + 1) * P],
                            rhs=wv_t[:, kc, :],
                            start=(kc == 0), stop=(kc == KC - 1),
                        )
                    nc.vector.tensor_tensor(
                        V1[:, sc, :, 0:64], ps[:, 0:4, :], bv_t[:], add)
                return f

            ysbs = {}

            def out_step(sc, nt, use_psA=False):
                # half of the partial out-projection for seq rows sc*128..:
                # 2 accumulating MMs (contraction over this core's 256 attn
                # dims) + bias add; DMA the row-chunk out after nt==1.
                def f():
                    if sc not in ysbs:
                        ysbs[sc] = yp.tile([P, E], BF, tag="ysb",
                                           name=f"ysb{sc}")
                    if use_psA:
                        ps = psA.tile([P, 1024], F32, tag="big",
                                      name=f"ops{sc}_{nt}")[:, 0:512]
                    else:
                        ps = psF.tile([P, 512], F32, tag="acc",
                                      name=f"ops{sc}_{nt}")
                    for cc in range(2):
                        nc.tensor.matmul(
                            ps[:],
                            lhsT=OT[:, cc, sc * P:(sc + 1) * P],
                            rhs=wo_t[:, cc, nt * 512:(nt + 1) * 512],
                            start=(cc == 0), stop=(cc == 1),
                        )
                    nc.vector.tensor_tensor(
                        ysbs[sc][:, nt * 512:(nt + 1) * 512], ps[:],
                        bo_t[:, nt * 512:(nt + 1) * 512], add)
                    if nt == 1:
                        nc.gpsimd.dma_start(
                            d_y[sc * P:(sc + 1) * P, :], ysbs.pop(sc)[:])
                return f

            # ---- prologue: K(c0,span0) + Q(c0,span0) only ----
            for f in qk_steps(wk_t, bk_t, KT, xks[0], 0, 0):
                f()
            for f in qk_steps(wq_t, bq_t, QT, xqs[0], 0, 0):
                f()

            # ---- attention units: 8 batches of 2 jc; trailing PVs and the
            # normalize of each unit are carried into the NEXT unit's first
            # batch so the exp stream never gaps at unit boundaries ----
            pend = []          # [(j, pt, c, pO0, pO1), ...] cross-unit
            carry = [None]     # pending normalize closure

            def emit_pv(j, pt, c, pO0, pO1):
                for hp, pO in ((0, pO0), (1, pO1)):
                    nc.tensor.matmul(
                        pO[:], lhsT=V1[:, j, 2 * c + hp, :],
                        rhs=pt[:, hp * 512:(hp + 1) * 512],
                        start=(j == 0), stop=(j == JC - 1),
                    )

            def normalize(c, t, pO0, pO1, mid=None):
                # OT_h = pO[0:64] / pO[64] (row 64 = sum of P). 1/S is
                # broadcast across partitions by GpSimd, off the PE FIFO.
                def f():
                    tsl = slice(t * 512, (t + 1) * 512)
                    osb = sm.tile([P, 1024], F32, tag="osb")
                    nc.vector.tensor_copy(osb[0:65, 0:512], pO0[:])
                    nc.vector.tensor_copy(osb[0:65, 512:1024], pO1[:])
                    if mid is not None:
                        mid()
                    rec = sm.tile([1, 1024], F32, tag="rec")
                    nc.sync.dma_start(rec[:], osb[64:65, :])
                    rin = sm.tile([1, 1024], F32, tag="rin")
                    nc.vector.reciprocal_approx_fast(rin[:], rec[:])
                    rbs = sm.tile([64, 1024], F32, tag="rbs")
                    nc.gpsimd.partition_broadcast(rbs[:], rin[:])
                    r0, r1 = rbs[:, 0:512], rbs[:, 512:1024]
                    # head0: partitions align -> multiply straight into OT
                    nc.vector.tensor_tensor(
                        OT[0:64, c, tsl], osb[0:64, 0:512], r0, mult)
                    # head1: stage bf16 then partition-shift DMA
                    ott = sm.tile([64, 512], BF, tag="ott")
                    nc.vector.tensor_tensor(
                        ott[:], osb[0:64, 512:1024], r1, mult)
                    nc.sync.dma_start(OT[64:128, c, tsl], ott[:])
                return f

            def attn_unit(c, t, fillers):
                # fillers: list of 8 lists of callables (one per batch)
                tsl = slice(t * 512, (t + 1) * 512)
                pO0 = psO.tile([65, 512], F32, tag="pO", name=f"pO0_{c}{t}")
                pO1 = psO.tile([65, 512], F32, tag="pO", name=f"pO1_{c}{t}")

                for b in range(8):
                    for j in (2 * b, 2 * b + 1):
                        jsl = slice(j * P, (j + 1) * P)
                        pL = psA.tile([P, 1024], F32, tag="big",
                                      name=f"pL{c}{t}_{j}")
                        nc.tensor.matmul(
                            pL[:, 0:512],
                            lhsT=KT[0:64, c, jsl], rhs=QT[0:64, c, tsl],
                            start=True, stop=True,
                        )
                        nc.tensor.matmul(
                            pL[:, 512:1024],
                            lhsT=KT[64:128, c, jsl], rhs=QT[64:128, c, tsl],
                            start=True, stop=True,
                        )
                        pt = ptp.tile([P, 1024], BF, tag="pt")
                        nc.scalar.activation(pt[:], pL[:], Exp)
                        pend.append((j, pt, c, pO0, pO1))
                    while len(pend) > 2:
                        emit_pv(*pend.pop(0))
                    if b == 0 and carry[0] is not None:
                        normalize(*carry[0])()
                        carry[0] = None
                    for f in fillers[b]:
                        f()
                carry[0] = (c, t, pO0, pO1)

            # unit 0 = (0,0): V chunks (2/batch) + K(c0,s1-3) + Q(c0,s1)
            u0 = [[] for _ in range(8)]
            for b in range(8):
                u0[b] += [v_chunk(2 * b), v_chunk(2 * b + 1)]
            for b, (kind, sp) in ((0, ('K', 1)), (2, ('K', 2)),
                                  (4, ('K', 3)), (6, ('Q', 1))):
                w, bb, dst, xs = ((wk_t, bk_t, KT, xks[sp]) if kind == 'K'
                                  else (wq_t, bq_t, QT, xqs[sp]))
                steps = qk_steps(w, bb, dst, xs, 0, sp)
                u0[b] = steps[:4] + u0[b]

            # units 1-4: remaining projections, 2 steps per batch
            def spread(unit, b0, kind, c, sp):
                w, bb, dst, xs = ((wk_t, bk_t, KT, xks[sp]) if kind == 'K'
                                  else (wq_t, bq_t, QT, xqs[sp]))
                steps = qk_steps(w, bb, dst, xs, c, sp)
                unit[b0] += steps[0:2]
                unit[b0 + 1] += steps[2:4]

            u1 = [[] for _ in range(8)]
            spread(u1, 0, 'K', 1, 0)
            spread(u1, 2, 'K', 1, 1)
            spread(u1, 4, 'Q', 0, 2)
            u2 = [[] for _ in range(8)]
            spread(u2, 0, 'K', 1, 2)
            spread(u2, 2, 'K', 1, 3)
            spread(u2, 4, 'Q', 0, 3)
            spread(u2, 6, 'Q', 1, 0)
            u3 = [[] for _ in range(8)]
            spread(u3, 0, 'Q', 1, 1)
            spread(u3, 2, 'Q', 1, 2)
            u4 = [[] for _ in range(8)]
            spread(u4, 0, 'Q', 1, 3)

            def outfill(t):
                # 8 steps placed in batches 1-6 so the boundary batches
                # (0 and 7, which carry the normalize and trailing PVs)
                # stay light on the PE and DVE
                s = [out_step(sc, nt)
                     for sc in range(4 * t, 4 * t + 4) for nt in range(2)]
                return [[], [s[0], s[1]], [s[2]], [s[3]],
                        [s[4], s[5]], [s[6]], [s[7]], []]

            attn_unit(0, 0, u0)
            attn_unit(0, 1, u1)
            attn_unit(0, 2, u2)
            attn_unit(0, 3, u3)
            attn_unit(1, 0, u4)
            attn_unit(1, 1, outfill(0))
            attn_unit(1, 2, outfill(1))
            attn_unit(1, 3, outfill(2))
            # epilogue: flush the carried PVs, then run the last unit's
            # normalize with the final out-projection split in two phases:
            # the cc=0 half (depends only on long-finished c=0 OT) starts
            # right after the pO copies, on all 8 freed PSUM banks, hiding
            # under the reciprocal/broadcast chain; the cc=1 half follows
            # once the last OT columns land.
            while pend:
                emit_pv(*pend.pop(0))
            tpairs = [(sc, nt) for sc in range(12, 16) for nt in range(2)]
            taccs = {}

            def cc0_mm(acc, sc, nt):
                nc.tensor.matmul(
                    acc, lhsT=OT[:, 0, sc * P:(sc + 1) * P],
                    rhs=wo_t[:, 0, nt * 512:(nt + 1) * 512],
                    start=True, stop=False,
                )

            # phase A1: the c=0 contraction halves (only depend on OT c=0,
            # finished three units ago) start immediately after the trailing
            # PVs on the freed psA banks — keeps the PE warm and busy while
            # the normalize chain (DVE/DMA) computes 1/S
            big0 = psA.tile([P, 1024], F32, tag="big", name="ta0")
            big1 = psA.tile([P, 1024], F32, tag="big", name="ta1")
            for i, (sc, nt) in enumerate(tpairs[:4]):
                acc = [big0[:, 0:512], big0[:, 512:1024],
                       big1[:, 0:512], big1[:, 512:1024]][i]
                taccs[(sc, nt)] = acc
                cc0_mm(acc, sc, nt)

            for i, (sc, nt) in enumerate(tpairs[6:]):
                acc = psF.tile([P, 512], F32, tag="acc", name=f"tf{i}")[:]
                taccs[(sc, nt)] = acc
                cc0_mm(acc, sc, nt)

            def tail_mid():
                # phase A2: two more accs on the pO banks freed by the copies
                for i, (sc, nt) in enumerate(tpairs[4:6]):
                    acc = psO.tile([P, 512], F32, tag="pO",
                                   name=f"to{i}")[:]
                    taccs[(sc, nt)] = acc
                    cc0_mm(acc, sc, nt)

            normalize(*carry[0], mid=tail_mid)()
            carry[0] = None
            # phase B: the c=1 halves as the last OT columns land
            for sc, nt in tpairs:
                acc = taccs[(sc, nt)]
                nc.tensor.matmul(
                    acc, lhsT=OT[:, 1, sc * P:(sc + 1) * P],
                    rhs=wo_t[:, 1, nt * 512:(nt + 1) * 512],
                    start=False, stop=True,
                )
                if sc not in ysbs:
                    ysbs[sc] = yp.tile([P, E], BF, tag="ysb",
                                       name=f"ysbt{sc}")
                nc.vector.tensor_tensor(
                    ysbs[sc][:, nt * 512:(nt + 1) * 512], acc,
                    bo_t[:, nt * 512:(nt + 1) * 512], add)
                if nt == 1:
                    nc.gpsimd.dma_start(
                        d_y[sc * P:(sc + 1) * P, :], ysbs.pop(sc)[:])

    nc.compile()
    return nc


def _get_program():
    global _NC
    if _NC is None:
        _NC = _build_program()
    return _NC


def kernel(q, k, v, Wq, bq, Wk, bk, Wv, bv, Wo, bo):
    from concourse.bass_utils import run_bass_kernel_spmd

    q = np.asarray(q, np.float32)
    k = np.asarray(k, np.float32)
    v = np.asarray(v, np.float32)
    Wq = np.asarray(Wq, np.float32)
    Wk = np.asarray(Wk, np.float32)
    Wv = np.asarray(Wv, np.float32)
    Wo = np.asarray(Wo, np.float32)
    bq = np.asarray(bq, np.float32)
    bk = np.asarray(bk, np.float32)
    bv = np.asarray(bv, np.float32)
    bo = np.asarray(bo, np.float32)

    nc = _get_program()

    def tile_qk(xb):
        # [S, E] -> x^T tiled [4, 128, KC, 512]
        return np.ascontiguousarray(
            xb.T.reshape(KC, P, 4, 512).transpose(2, 1, 0, 3)).astype(BF16)

    def tile_v(xb):
        # [S, E] -> x^T tiled [8, 128, KC, 256]
        return np.ascontiguousarray(
            xb.T.reshape(KC, P, 8, GE).transpose(2, 1, 0, 3)).astype(BF16)

    xT = {"xqT": [tile_qk(q[b]) for b in range(2)],
          "xkT": [tile_qk(k[b]) for b in range(2)],
          "xvT": [tile_v(v[b]) for b in range(2)]}

    def wprep(W, scale=1.0):
        # [E, GE] slice -> [P, KC, GE] partition-major
        return [
            np.ascontiguousarray(
                (W[:, g * GE:(g + 1) * GE] * scale)
                .reshape(KC, P, GE).transpose(1, 0, 2)
            ).astype(BF16)
            for g in range(4)
        ]

    wq_g = wprep(Wq, 0.125)
    wk_g = wprep(Wk)
    wv_g = wprep(Wv)
    wo_g = [
        np.ascontiguousarray(
            Wo[g * GE:(g + 1) * GE, :].reshape(2, P, E).transpose(1, 0, 2)
        ).astype(BF16)
        for g in range(4)
    ]
    bq_g = [np.ascontiguousarray((bq[g * GE:(g + 1) * GE] * 0.125)
                                 .reshape(2, P).T).astype(np.float32)
            for g in range(4)]
    bk_g = [np.ascontiguousarray(bk[g * GE:(g + 1) * GE].reshape(2, P).T)
            .astype(np.float32) for g in range(4)]
    bv_g = [np.ascontiguousarray(np.broadcast_to(
        bv[g * GE:(g + 1) * GE].astype(np.float32),
        (P, GE))).reshape(P, 4, 64) for g in range(4)]
    bo_full = np.ascontiguousarray(
        np.broadcast_to(bo.astype(np.float32), (P, E)))
    bo_zero = np.zeros((P, E), np.float32)

    in_maps = []
    for c in range(NCORES):
        b, g = divmod(c, 4)
        in_maps.append({
            "xqT": xT["xqT"][b],
            "xkT": xT["xkT"][b],
            "xvT": xT["xvT"][b],
            "wq": wq_g[g], "wk": wk_g[g], "wv": wv_g[g], "wo": wo_g[g],
            "bqs": bq_g[g], "bks": bk_g[g], "bvb": bv_g[g],
            "bob": bo_full if g == 0 else bo_zero,
        })

    res = run_bass_kernel_spmd(nc, in_maps, list(range(NCORES)),
                               **_RUN_KWARGS)
    globals()["LAST_RESULTS"] = res

    parts = [np.asarray(res.results[c]["y"], np.float32)
             for c in range(NCORES)]
    out = np.stack([
        parts[0] + parts[1] + parts[2] + parts[3],
        parts[4] + parts[5] + parts[6] + parts[7],
    ]).astype(np.float32)
    return out


# test-harness hooks (kernel.py itself never enables tracing)
_RUN_KWARGS = {}
LAST_RESULTS = None
